# revision 20
# baseline (speedup 1.0000x reference)
"""Trainium2 Bass kernel for the tanh-RNN problem.

Math (per reference):
  xp = x @ W2 + b                      # [B, T, H]
  h_{t+1} = tanh(h_t @ W1 + xp_t),  h_0 = 0;  return h_T   # [B, H]

Shapes: B=64, T=2048, D=H=256, fp32.

Distribution: data-parallel over batch, 8 rows per core on 8 NeuronCores,
weights replicated, no collectives; host shards inputs / gathers outputs.

Per-core design (everything kept in "transposed" H-on-partitions layout so
the serial recurrence needs no per-step transposes):

Phase 1 (fp32): stream x in, PE-transpose [t, d] -> [d, t] tiles via
  identity matmuls, GEMM with W2 128x128 blocks into PSUM, then evict with
  bias add and an fp16 hi/lo split:
      tmp = psum + b;  xp_hi = fp16(tmp);  xp_lo = fp16(tmp - xp_hi)
  stored as [128p, (c, b, t)] where H = c*128 + p.
  The first t-window is emitted upfront; the rest is chopped into
  single-instruction closures and drip-fed between recurrence steps so it
  executes inside the recurrence's idle engine slots.

Phase 2: 2048 serial steps, all matmuls in fp16 with hi/lo error
  compensation (h @ W1 = h_hi@W1_hi + h_hi@W1_lo + h_lo@W1_hi, ~1e-6
  end-to-end, fp32-grade). fp16 matmul+ldweights pairs issue at ~27 ns vs
  fp32's ~426 ns (fp32 runs LOW/HIGH dual passes with 333 ns weight loads).
  Per step one PSUM bank tile [128, 16] (cols = c*8 + b):
    - xp_hi, xp_lo enter via identity-stationary matmuls (2 pairs)
    - 8 hi-pairs (W_hi, W_lo x h_hi) + 4 lo-pairs (W_hi x h_lo)
    - ACT: h_f32 = tanh(psum)
    - DVE: h_hi = fp16(h_f32); h_lo = fp16(h_f32 - h_hi)  (back-to-back)
  Serial chain per step: ACT -> cast -> sub -> 4 lo-pairs -> close, ~1.1 us;
  the hi-pairs and xp entry hide under the chain.
"""

from collections import deque

import numpy as np

import bass_rust
import concourse.bacc as bacc
import concourse.bass as bass
import concourse.mybir as mybir
import concourse.tile as tile
from concourse import bass_utils, masks

N_CORES = 8
B = 64          # full batch
B_LOC = 8       # batch rows per core
T = 2048
D = 256
H = 256
TB = 512        # phase-1 t-block (one full PSUM bank at fp32)
F32 = mybir.dt.float32
F16 = mybir.dt.float16
BF16 = mybir.dt.bfloat16
Tanh = mybir.ActivationFunctionType.Tanh
Sub = mybir.AluOpType.subtract


def build_rnn_kernel(nc, tc, t_len=T, tb=TB):
    x = nc.dram_tensor("x", [B_LOC, t_len, D], F32, kind="ExternalInput")
    w1d = nc.dram_tensor("W1", [H, H], F32, kind="ExternalInput")
    w2d = nc.dram_tensor("W2", [D, H], F32, kind="ExternalInput")
    bd = nc.dram_tensor("b", [H], F32, kind="ExternalInput")
    outd = nc.dram_tensor("out", [B_LOC, H], F32, kind="ExternalOutput")

    nsub = tb // 128          # 128-row subtiles per t-block
    nblk = t_len // tb        # t-blocks

    consts = tc.alloc_tile_pool(name="consts", bufs=1)
    ident = consts.tile([128, 128], F32, tag="ident", name="ident")
    masks.make_identity(nc, ident[:])
    ident16 = consts.tile([128, 128], F16, tag="ident16", name="ident16")
    nc.vector.tensor_copy(ident16[:], ident[:])

    # W1 128x128 blocks as bf16 hi/lo pairs (consumed against the bf16
    # bit-view of h_f32) plus fp16 copies of the same values (consumed
    # against the fp16 h_lo residual).
    w1f = [[consts.tile([128, 128], F32, tag=f"w1f_{ki}{mj}",
                        name=f"w1f_{ki}{mj}") for mj in range(2)]
           for ki in range(2)]
    w1h = [[consts.tile([128, 128], BF16, tag=f"w1h_{ki}{mj}",
                        name=f"w1h_{ki}{mj}") for mj in range(2)]
           for ki in range(2)]
    w1l = [[consts.tile([128, 128], BF16, tag=f"w1l_{ki}{mj}",
                        name=f"w1l_{ki}{mj}") for mj in range(2)]
           for ki in range(2)]
    w1h16 = [[consts.tile([128, 128], F16, tag=f"w1h16_{ki}{mj}",
                          name=f"w1h16_{ki}{mj}") for mj in range(2)]
             for ki in range(2)]
    w1l16 = [[consts.tile([128, 128], F16, tag=f"w1l16_{ki}{mj}",
                          name=f"w1l16_{ki}{mj}") for mj in range(2)]
             for ki in range(2)]
    w2t = [[consts.tile([128, 128], F32, tag=f"w2_{ki}{mj}",
                        name=f"w2_{ki}{mj}") for mj in range(2)]
           for ki in range(2)]
    for ki in range(2):
        for mj in range(2):
            nc.sync.dma_start(
                w1f[ki][mj][:],
                w1d.ap()[128 * ki:128 * (ki + 1), 128 * mj:128 * (mj + 1)])
            nc.sync.dma_start(
                w2t[ki][mj][:],
                w2d.ap()[128 * ki:128 * (ki + 1), 128 * mj:128 * (mj + 1)])
            nc.vector.tensor_copy(w1h[ki][mj][:], w1f[ki][mj][:])
            nc.vector.tensor_tensor(w1l[ki][mj][:], w1f[ki][mj][:],
                                    w1h[ki][mj][:], Sub)
            nc.vector.tensor_copy(w1h16[ki][mj][:], w1h[ki][mj][:])
            nc.vector.tensor_copy(w1l16[ki][mj][:], w1l[ki][mj][:])
    b_sb = consts.tile([128, 2], F32, tag="b_sb", name="b_sb")
    nc.sync.dma_start(b_sb[:], bd.ap().rearrange("(c p) -> p c", c=2))

    # Persistent xp buffers (fp16 hi/lo): [128, c * b * t], H = c*128 + p.
    xp_pool = tc.alloc_tile_pool(name="xp", bufs=1)
    xph = xp_pool.tile([128, 2 * B_LOC * t_len], F16, tag="xph", name="xph")
    xpl = xp_pool.tile([128, 2 * B_LOC * t_len], F16, tag="xpl", name="xpl")
    xphv = xph[:].rearrange("p (c b t) -> p c b t", c=2, b=B_LOC)
    xplv = xpl[:].rearrange("p (c b t) -> p c b t", c=2, b=B_LOC)

    # Phase-1 pools (coexist with phase-2: 2+2 PSUM banks here + 4 ps = 8)
    p1s = tc.alloc_tile_pool(name="p1_sbuf", bufs=2)
    p1p = tc.alloc_tile_pool(name="p1_psum", bufs=1, space="PSUM")

    def dma_gen(blk, b):
        """Load x rows for (blk, b) into a fresh xt tile (Sync engine only,
        no PE time). Returns the tile via the first yield."""
        t0 = blk * tb
        xt = p1s.tile([128, nsub * D], F32, tag="xt", name="xt", bufs=3)
        for i in range(nsub):
            nc.sync.dma_start(
                xt[:, i * D:(i + 1) * D],
                x.ap()[b, t0 + 128 * i:t0 + 128 * (i + 1), :])
            yield xt

    def comp_gen(blk, b, xt, mm_n):
        """Compute xp for (blk, b) from a pre-loaded xt tile, one
        instruction per next() so it can drip into phase-2 idle slots."""
        t0 = blk * tb
        tp = [p1p.tile([128, tb], F32, tag=f"tp{c}", name=f"tp{c}")
              for c in range(2)]
        for i in range(nsub):
            for dc in range(2):
                yield nc.tensor.transpose(
                    tp[dc][:, 128 * i:128 * (i + 1)],
                    xt[:, i * D + 128 * dc: i * D + 128 * (dc + 1)],
                    ident[:])
        xT = [p1s.tile([128, tb], F32, tag=f"xT{c}", name=f"xT{c}")
              for c in range(2)]
        for dc in range(2):
            yield nc.scalar.copy(xT[dc][:], tp[dc][:])
        xps = [p1p.tile([128, tb], F32, tag=f"xps{mj}", name=f"xps{mj}")
               for mj in range(2)]
        for mj in range(2):
            for ci in range(tb // mm_n):
                for ki in range(2):
                    yield nc.tensor.matmul(
                        xps[mj][:, mm_n * ci:mm_n * (ci + 1)],
                        w2t[ki][mj][:],
                        xT[ki][:, mm_n * ci:mm_n * (ci + 1)],
                        start=(ci == 0 and ki == 0),
                        stop=(ci == tb // mm_n - 1 and ki == 1))
        for mj in range(2):
            tmp = p1s.tile([128, tb], F32, tag=f"tmp{mj}", name=f"tmp{mj}")
            dh = xphv[:, mj, b, t0:t0 + tb]
            dl = xplv[:, mj, b, t0:t0 + tb]
            yield nc.vector.tensor_scalar_add(tmp[:], xps[mj][:],
                                              b_sb[:, mj:mj + 1])
            yield nc.scalar.copy(dh, tmp[:])
            yield nc.vector.tensor_tensor(dl, tmp[:], dh, Sub)

    # First window upfront (large matmuls); the rest drip-fed into phase 2.
    # DMAs run one block ahead of their compute so the dripped transposes
    # never stall the PE stream waiting on HBM.
    blocks = [(blk, b) for blk in range(1, nblk) for b in range(B_LOC)]
    for b in range(B_LOC):
        xt0 = None
        for xt0 in dma_gen(0, b):
            pass
        for _ in comp_gen(0, b, xt0, min(512, tb)):
            pass

    def drip():
        xt_next = None
        if blocks:
            for xt_next in dma_gen(*blocks[0]):
                yield None
        for i, (blk, b) in enumerate(blocks):
            xt_cur, xt_next = xt_next, None
            dma_it = iter(dma_gen(*blocks[i + 1])) if i + 1 < len(blocks) \
                else iter(())
            for inst in comp_gen(blk, b, xt_cur, 64):
                xt_next = next(dma_it, xt_next)
                yield inst

    work = deque([drip()])

    # ---------------- Phase 2: serial recurrence --------------------------
    with tc.tile_pool(name="p2_psum", bufs=4, space="PSUM") as p2p, \
         tc.tile_pool(name="p2_h", bufs=8) as p2h:
        h_hi = h_lo = h_f = None
        anchors = {}
        for t in range(t_len):
            if work:
                try:
                    inst = next(work[0])
                except StopIteration:
                    work.popleft()
                else:
                    if inst is not None and anchors:
                        eng = inst.ins.engine
                        anc = anchors.get(eng) or anchors.get("pe")
                        if anc is not None:
                            bass_rust.add_dep_helper(
                                inst.ins, anc.ins, sync=False,
                                reason="drip pacing")
            ps = p2p.tile([128, 16], F32, tag="ps", name="ps")
            mm0 = nc.tensor.matmul(ps[:], ident16[:], xphv[:, :, :, t],
                                   start=True, stop=False)
            nc.tensor.matmul(ps[:], ident16[:], xplv[:, :, :, t],
                             start=False, stop=(t == 0))
            if t > 0:
                mms = []
                for grp, hsrc in ((w1h, h_hi), (w1l, h_hi),
                                  (w1h16, h_lo), (w1l16, h_lo)):
                    for mj in range(2):
                        for ki in range(2):
                            mms.append((grp[ki][mj], hsrc, mj, ki))
                for idx, (w, hsrc, mj, ki) in enumerate(mms):
                    nc.tensor.matmul(ps[:, 8 * mj:8 * mj + 8], w[:],
                                     hsrc[:, 8 * ki:8 * ki + 8],
                                     start=False, stop=(idx == len(mms) - 1))
            h_f = p2h.tile([128, 16], F32, tag="hf", name="hf")
            act_i = nc.scalar.activation(h_f[:], ps[:], Tanh)
            # h_hi: free bf16 view of h_f's high 16 bits (truncation)
            h_hi = h_f[:].bitcast(BF16).rearrange(
                "p (n two) -> p n two", two=2)[:, :, 1]
            h_lo = p2h.tile([128, 16], F16, tag="hl", name="hl")
            sub_i = nc.vector.tensor_tensor(h_lo[:], h_f[:], h_hi, Sub)
            h_lo = h_lo[:]
            anchors = {act_i.ins.engine: act_i, sub_i.ins.engine: sub_i,
                       "pe": mm0}

        while work:
            for _ in work.popleft():
                pass

        # Final: h_last^T [128, 16] -> out [8, 256] (reuses a phase-1 bank)
        po = p1p.tile([16, 128], F32, tag="tp0", name="po")
        nc.tensor.transpose(po[:], h_f[:], ident[:])
        o_sb = p2h.tile([16, 128], F32, tag="o_sb", name="o_sb")
        nc.vector.tensor_copy(o_sb[:], po[:])
        for c in range(2):
            nc.sync.dma_start(outd.ap()[:, 128 * c:128 * (c + 1)],
                              o_sb[8 * c:8 * (c + 1), :])

    p1p.release()
    p1s.release()
    xp_pool.release()
    consts.release()


_NC_CACHE = {}


def _get_compiled(t_len=T, tb=TB, n_cores=N_CORES):
    key = (t_len, tb, n_cores)
    if key not in _NC_CACHE:
        nc = bacc.Bacc("TRN2", target_bir_lowering=False, debug=False,
                       num_devices=n_cores)
        with tile.TileContext(nc) as tc:
            build_rnn_kernel(nc, tc, t_len=t_len, tb=tb)
        nc.compile()
        _NC_CACHE[key] = nc
    return _NC_CACHE[key]


def kernel(x, W1, W2, b):
    x = np.ascontiguousarray(x, dtype=np.float32)
    W1 = np.ascontiguousarray(W1, dtype=np.float32)
    W2 = np.ascontiguousarray(W2, dtype=np.float32)
    b = np.ascontiguousarray(b, dtype=np.float32)

    nc = _get_compiled()
    in_maps = [
        {"x": x[i * B_LOC:(i + 1) * B_LOC], "W1": W1, "W2": W2, "b": b}
        for i in range(N_CORES)
    ]
    res = bass_utils.run_bass_kernel_spmd(nc, in_maps,
                                          core_ids=list(range(N_CORES)))
    return np.concatenate([res.results[i]["out"] for i in range(N_CORES)],
                          axis=0)
